# revision 9
# baseline (speedup 1.0000x reference)
"""CNN+GAT kernel for Trainium2, 8 NeuronCores, data-parallel over the batch.

Problem (hardcoded): B=16 graphs, L=384 timesteps, N=128 nodes, E=4096 edges.
Per graph: 4-layer 1D CNN (1->32->64->128->256, k=3 SAME, ReLU) over each
node's series, mean-pool over time, FC 256->256, then 3x (GATConv + GraphNorm
+ residual ReLU), mean-pool over nodes, linear classifier -> scalar.

Sharding: 2 graphs per core. Inside a core everything is computed per graph.

Implementation notes:
 - Conv layers run on the TensorEngine in bf16 with nodes interleaved along
   the free dim (col = (t+1)*32 + n within a 32-node chunk; one zero-padded
   timestep on each side), taps accumulated in PSUM.  conv1 packs its 3 taps
   into K and runs 4 node-chunks concurrently on diagonal 32x32 PE tiles;
   conv2 runs 4 chunks concurrently (K=32 each); conv3 two (K=64).
 - ReLU+bias happens on the ScalarEngine while copying PSUM->SBUF (bf16).
 - Time pooling is a strided VectorEngine tensor_reduce; the 1/384 is folded
   into the FC weight host-side.
 - The GAT edge softmax is computed densely: logitsT[j,i] = al_s[j]+al_d[i]
   on a [128 src, 4*128 dst] tile (outer sums via K=1 matmuls + per-partition
   bias in a Prelu activation), exp on ScalarE, multiplied by the edge
   multiplicity matrix cntT (host-built from edge_index, self-loops added).
   Messages and softmax denominators are matmuls with exT as the stationary
   operand.  GraphNorm statistics use ones-vector matmuls to reduce over
   nodes (partitions).
"""
import numpy as np
import ml_dtypes

B, L, N, E = 16, 384, 128, 4096
H, C, F = 4, 64, 256
EPS = 1e-5
GC = 32                 # nodes per conv chunk
TC = (L + 2) * GC       # padded cols per chunk = 12352
NSLICE = L * GC // 512  # 512-col psum slices per chunk = 24
GPC = 2                 # graphs per core
NCORES = 8

_BF16 = ml_dtypes.bfloat16
_cache = {}


def _build_program():
    import concourse.bacc as bacc
    import concourse.mybir as mybir
    import concourse.tile as tile

    F32 = mybir.dt.float32
    BF16 = mybir.dt.bfloat16
    AF = mybir.ActivationFunctionType
    ALU = mybir.AluOpType

    nc = bacc.Bacc("TRN2", target_bir_lowering=False, debug=False,
                   num_devices=NCORES)
    d = {}

    def par(name, shape, dt):
        d[name] = nc.dram_tensor(name, list(shape), dt, kind="ExternalInput")

    par("xprep", [GPC, 128, TC], BF16)
    par("wc1", [128, 32], BF16)
    for k in range(3):
        par(f"wc2k{k}", [128, 64], BF16)
        par(f"wc3k{k}", [128, 128], BF16)
        for m in range(2):
            par(f"wc4k{k}m{m}", [128, 128], BF16)
    for nm in ("bias1", "bias2", "bias3", "bias4a", "bias4b"):
        par(nm, [128, 1], F32)
    par("fcwT0", [128, 256], F32)
    par("fcwT1", [128, 256], F32)
    par("fcb_bc", [128, 256], F32)
    for l in range(3):
        par(f"wtT{l}t0", [128, 256], BF16)
        par(f"wtT{l}t1", [128, 256], BF16)
        par(f"wasad{l}t0", [128, 8], BF16)
        par(f"wasad{l}t1", [128, 8], BF16)
        par(f"gatb_bc{l}", [128, 256], F32)
        par(f"nb_bc{l}", [128, 256], F32)
        par(f"msrow{l}", [1, 256], F32)
        par(f"grow{l}", [1, 256], F32)
    par("cntT", [128, 128], F32)
    par("ones_col", [128, 1], F32)      # value 1/128
    par("ones_row_f", [1, 128], F32)    # value 1.0
    par("ones_row_bf", [1, 128], BF16)  # value 1.0
    par("ones_col_bf", [128, 1], BF16)  # value 1.0
    par("ident", [128, 128], F32)
    par("clsw", [1, 256], F32)
    par("clsb", [1, 1], F32)
    out_d = nc.dram_tensor("out", [1, GPC], F32, kind="ExternalOutput")

    with tile.TileContext(nc, pool_alloc_mode="queue") as tc:
        with tc.tile_pool(name="const", bufs=1) as cp:
            ct = {}
            for name, t in d.items():
                if name in ("xprep",):
                    continue
                shape = list(t.shape)
                dt = t.dtype
                ct[name] = cp.tile(shape, dt, tag=name, name=f"ct_{name}")
                nc.sync.dma_start(ct[name][:], t[:])

            out_sb = cp.tile([1, GPC], F32, tag="out_sb")
            dots = cp.tile([1, GPC], F32, tag="dots")

            for g in range(GPC):
                with tc.tile_pool(name=f"g{g}mid", bufs=1) as mp:
                    poolf = [mp.tile([128, 128], F32, tag=f"pool{m}", name=f"poolf{m}")
                             for m in range(2)]
                    # ---------------- conv1..conv4 ----------------
                    # one SBUF pool; slot "A" holds x then c2, slot "B" holds
                    # c1 then c3 (tag reuse = WAR-ordered slot recycling)
                    with tc.tile_pool(name=f"g{g}conv", bufs=1) as pc, \
                         tc.tile_pool(name=f"g{g}psA", bufs=6, space="PSUM") as psa:
                        xt = pc.tile([128, 2 * TC], BF16, tag="A", name="xt")
                        nc.sync.dma_start(xt[:, 0:TC], d["xprep"][g])
                        c1 = pc.tile([128, TC], BF16, tag="B", name="c1",
                                     padded_shape=[128, 4 * TC])
                        nc.vector.memset(c1[:, 0:GC], 0.0)
                        nc.vector.memset(c1[:, TC - GC:TC], 0.0)
                        # conv1: K=3 (taps stacked), 4 chunks on diagonal tiles
                        for s in range(NSLICE):
                            lo = GC + 512 * s
                            pt = psa.tile([128, 512], F32, tag="cps")
                            for j in range(4):
                                nc.tensor.matmul(
                                    pt[32 * j:32 * j + 32, :],
                                    ct["wc1"][32 * j:32 * j + 3, :],
                                    xt[32 * j:32 * j + 3, lo:lo + 512],
                                    start=True, stop=True,
                                    tile_position=(32 * j, 32 * j))
                            nc.scalar.activation(c1[:, lo:lo + 512], pt[:],
                                                 AF.Relu, bias=ct["bias1"][:])
                        c2 = pc.tile([128, 2 * TC], BF16, tag="A", name="c2")
                        for b in range(2):
                            nc.vector.memset(c2[:, b * TC:b * TC + GC], 0.0)
                            nc.vector.memset(c2[:, (b + 1) * TC - GC:(b + 1) * TC], 0.0)
                        # conv2: per-tap K=32, 4 chunks concurrent (2 psum tiles)
                        for s in range(NSLICE):
                            lo = GC + 512 * s
                            pts = [psa.tile([128, 512], F32, tag="cps", name=f"c2ps{i}")
                                   for i in range(2)]
                            for j in range(4):
                                pt = pts[j // 2]
                                ro = 64 * (j % 2)
                                for k in range(3):
                                    nc.tensor.matmul(
                                        pt[ro:ro + 64, :],
                                        ct[f"wc2k{k}"][32 * j:32 * j + 32, :],
                                        c1[32 * j:32 * j + 32,
                                           512 * s + GC * k:512 * s + GC * k + 512],
                                        start=(k == 0), stop=(k == 2),
                                        tile_position=(32 * j, ro))
                            for b in range(2):
                                nc.scalar.activation(
                                    c2[:, b * TC + lo:b * TC + lo + 512], pts[b][:],
                                    AF.Relu, bias=ct["bias2"][:])
                        c3 = pc.tile([128, 4 * TC], BF16, tag="B", name="c3")
                        for b in range(4):
                            nc.vector.memset(c3[:, b * TC:b * TC + GC], 0.0)
                            nc.vector.memset(c3[:, (b + 1) * TC - GC:(b + 1) * TC], 0.0)
                        # conv3: per-tap K=64; chunk j reads c2 rows 64*(j%2),
                        # col-block j//2; writes c3 col-block j (full 128 rows)
                        for blk in range(2):
                            for s in range(NSLICE):
                                lo = GC + 512 * s
                                pts = [psa.tile([128, 512], F32, tag="cps", name=f"c3ps{i}")
                                       for i in range(2)]
                                for half in range(2):
                                    j = 2 * blk + half
                                    ro = 64 * half
                                    for k in range(3):
                                        nc.tensor.matmul(
                                            pts[half][:, :],
                                            ct[f"wc3k{k}"][ro:ro + 64, :],
                                            c2[ro:ro + 64,
                                               blk * TC + 512 * s + GC * k:
                                               blk * TC + 512 * s + GC * k + 512],
                                            start=(k == 0), stop=(k == 2),
                                            tile_position=(ro, 0))
                                    nc.scalar.activation(
                                        c3[:, j * TC + lo:j * TC + lo + 512],
                                        pts[half][:], AF.Relu, bias=ct["bias3"][:])
                        # -------- conv4 + slice-wise time pool --------
                        for j in range(4):
                            for m in range(2):
                                partials = pc.tile([128, 768], F32, tag="pp",
                                                   bufs=2, name="partials")
                                for s in range(NSLICE):
                                    pt = psa.tile([128, 512], F32, tag="cps",
                                                  name="c4pt")
                                    for k in range(3):
                                        nc.tensor.matmul(
                                            pt[:],
                                            ct[f"wc4k{k}m{m}"][:],
                                            c3[:, j * TC + 512 * s + GC * k:
                                                  j * TC + 512 * s + GC * k + 512],
                                            start=(k == 0), stop=(k == 2))
                                    c4sl = pc.tile([128, 512], BF16, tag="c4sl",
                                                   bufs=4, name="c4sl")
                                    nc.scalar.activation(
                                        c4sl[:], pt[:], AF.Relu,
                                        bias=ct["bias4a" if m == 0 else "bias4b"][:])
                                    nc.vector.tensor_reduce(
                                        partials[:, 32 * s:32 * s + 32],
                                        c4sl[:].rearrange("p (t n) -> p n t", n=GC),
                                        axis=mybir.AxisListType.X, op=ALU.add)
                                nc.vector.tensor_reduce(
                                    poolf[m][:, GC * j:GC * j + GC],
                                    partials[:].rearrange("p (s n) -> p n s", n=GC),
                                    axis=mybir.AxisListType.X, op=ALU.add)
                    # ---------------- FC + GAT ----------------
                    with tc.tile_pool(name=f"g{g}gat", bufs=1) as gp, \
                         tc.tile_pool(name=f"g{g}gatx", bufs=2) as gx, \
                         tc.tile_pool(name=f"g{g}psC", bufs=1, space="PSUM") as psc:
                        fc_ps = psc.tile([128, 256], F32, tag="ps1")
                        for m in range(2):
                            nc.tensor.matmul(fc_ps[:], poolf[m][:],
                                             ct[f"fcwT{m}"][:],
                                             start=(m == 0), stop=(m == 1))
                        X = gx.tile([128, 256], F32, tag="X")
                        nc.vector.tensor_tensor(X[:], fc_ps[:], ct["fcb_bc"][:],
                                                op=ALU.add)

                        for l in range(3):
                            xfm_ps = psc.tile([128, 256], F32, tag="ps1")
                            for t in range(2):
                                nc.tensor.transpose(
                                    xfm_ps[:, 128 * t:128 * t + 128],
                                    X[:, 128 * t:128 * t + 128], ct["ident"][:])
                            xfm_bf = gp.tile([128, 256], BF16, tag="xfm")
                            nc.vector.tensor_copy(xfm_bf[:], xfm_ps[:])

                            h_ps = psc.tile([128, 256], F32, tag="ps2")
                            alnm_ps = psc.tile([128, 8], F32, tag="ps3")
                            aldf_ps = psc.tile([1, 512], F32, tag="ps4")
                            for t in range(2):
                                nc.tensor.matmul(h_ps[:],
                                                 xfm_bf[:, 128 * t:128 * t + 128],
                                                 ct[f"wtT{l}t{t}"][:],
                                                 start=(t == 0), stop=(t == 1))
                                nc.tensor.matmul(alnm_ps[:],
                                                 xfm_bf[:, 128 * t:128 * t + 128],
                                                 ct[f"wasad{l}t{t}"][:],
                                                 start=(t == 0), stop=(t == 1))
                                for hh in range(4):
                                    nc.tensor.matmul(
                                        aldf_ps[0:1, 128 * hh:128 * hh + 128],
                                        ct[f"wasad{l}t{t}"][:, 4 + hh:5 + hh],
                                        xfm_bf[:, 128 * t:128 * t + 128],
                                        start=(t == 0), stop=(t == 1))
                            hnm_bf = gp.tile([128, 256], BF16, tag="hnm")
                            nc.vector.tensor_copy(hnm_bf[:], h_ps[:])
                            alnm = gp.tile([128, 8], F32, tag="alnm")
                            nc.vector.tensor_copy(alnm[:], alnm_ps[:])
                            aldf = gp.tile([1, 512], BF16, tag="aldf")
                            nc.vector.tensor_copy(aldf[:], aldf_ps[:])

                            lg_ps = psc.tile([128, 512], F32, tag="ps5")
                            for hh in range(4):
                                nc.tensor.matmul(
                                    lg_ps[:, 128 * hh:128 * hh + 128],
                                    ct["ones_row_bf"][:],
                                    aldf[0:1, 128 * hh:128 * hh + 128],
                                    start=True, stop=True)
                            # leaky(lg + al_s) then exp, then * cntT
                            lr = gp.tile([128, 512], F32, tag="lr")
                            al02 = gp.tile([128, 1], F32, tag="al02")
                            nc.vector.memset(al02[:], 0.2)
                            for hh in range(4):
                                nc.scalar.activation(
                                    lr[:, 128 * hh:128 * hh + 128],
                                    lg_ps[:, 128 * hh:128 * hh + 128],
                                    AF.Prelu, bias=alnm[:, hh:hh + 1],
                                    scale=1.0, alpha=al02[:])
                            ex = gp.tile([128, 512], F32, tag="ex")
                            nc.scalar.activation(ex[:], lr[:], AF.Exp)
                            exT = gp.tile([128, 512], BF16, tag="exT")
                            for hh in range(4):
                                nc.vector.tensor_tensor(
                                    exT[:, 128 * hh:128 * hh + 128],
                                    ex[:, 128 * hh:128 * hh + 128],
                                    ct["cntT"][:], op=ALU.mult)

                            msg_ps = psc.tile([128, 256], F32, tag="ps1")
                            s_ps = psc.tile([128, 4], F32, tag="ps2")
                            for hh in range(4):
                                nc.tensor.matmul(
                                    msg_ps[:, 64 * hh:64 * hh + 64],
                                    exT[:, 128 * hh:128 * hh + 128],
                                    hnm_bf[:, 64 * hh:64 * hh + 64],
                                    start=True, stop=True)
                                nc.tensor.matmul(
                                    s_ps[:, hh:hh + 1],
                                    exT[:, 128 * hh:128 * hh + 128],
                                    ct["ones_col_bf"][:],
                                    start=True, stop=True)
                            r2 = gp.tile([128, 4], F32, tag="r2")
                            nc.vector.reciprocal(r2[:], s_ps[:])
                            y = gp.tile([128, 256], F32, tag="y")
                            for hh in range(4):
                                nc.vector.scalar_tensor_tensor(
                                    y[:, 64 * hh:64 * hh + 64],
                                    msg_ps[:, 64 * hh:64 * hh + 64],
                                    r2[:, hh:hh + 1],
                                    ct[f"gatb_bc{l}"][:, 64 * hh:64 * hh + 64],
                                    op0=ALU.mult, op1=ALU.add)
                            # GraphNorm
                            mu_ps = psc.tile([1, 256], F32, tag="ps4")
                            nc.tensor.matmul(mu_ps[:], ct["ones_col"][:], y[:],
                                             start=True, stop=True)
                            msmu = gp.tile([1, 256], F32, tag="msmu")
                            nc.vector.tensor_tensor(msmu[:], mu_ps[:],
                                                    ct[f"msrow{l}"][:], op=ALU.mult)
                            msmub_ps = psc.tile([128, 256], F32, tag="ps5")
                            nc.tensor.matmul(msmub_ps[:], ct["ones_row_f"][:],
                                             msmu[:], start=True, stop=True)
                            o = gp.tile([128, 256], F32, tag="o")
                            nc.vector.tensor_tensor(o[:], y[:], msmub_ps[:],
                                                    op=ALU.subtract)
                            sq = gp.tile([128, 256], F32, tag="sq")
                            nc.vector.tensor_tensor(sq[:], o[:], o[:], op=ALU.mult)
                            var_ps = psc.tile([1, 256], F32, tag="ps3")
                            nc.tensor.matmul(var_ps[:], ct["ones_col"][:], sq[:],
                                             start=True, stop=True)
                            # rstd = exp(-0.5 * ln(var + eps)); Ln+Exp share a
                            # table set (no ACT table churn with conv Relu/Exp)
                            epsc = gp.tile([1, 1], F32, tag="epsc")
                            nc.vector.memset(epsc[:], EPS)
                            lnv = gp.tile([1, 256], F32, tag="lnv")
                            nc.scalar.activation(lnv[:], var_ps[:], AF.Ln,
                                                 bias=epsc[:], scale=1.0)
                            rstd = gp.tile([1, 256], F32, tag="rstd")
                            nc.scalar.activation(rstd[:], lnv[:], AF.Exp,
                                                 scale=-0.5)
                            gs = gp.tile([1, 256], F32, tag="gs")
                            nc.vector.tensor_tensor(gs[:], rstd[:],
                                                    ct[f"grow{l}"][:], op=ALU.mult)
                            gsb_ps = psc.tile([128, 256], F32, tag="ps2")
                            nc.tensor.matmul(gsb_ps[:], ct["ones_row_f"][:],
                                             gs[:], start=True, stop=True)
                            t1 = gp.tile([128, 256], F32, tag="t1")
                            nc.vector.tensor_tensor(t1[:], o[:], gsb_ps[:],
                                                    op=ALU.mult)
                            t2 = gp.tile([128, 256], F32, tag="t2")
                            nc.vector.tensor_tensor(t2[:], t1[:], X[:], op=ALU.add)
                            t3 = gp.tile([128, 256], F32, tag="t3")
                            nc.vector.tensor_tensor(t3[:], t2[:],
                                                    ct[f"nb_bc{l}"][:], op=ALU.add)
                            X = gx.tile([128, 256], F32, tag="X")
                            nc.vector.tensor_scalar_max(X[:], t3[:], 0.0)

                        pooled_ps = psc.tile([1, 256], F32, tag="ps1")
                        nc.tensor.matmul(pooled_ps[:], ct["ones_col"][:], X[:],
                                         start=True, stop=True)
                        scr = gp.tile([1, 256], F32, tag="scr")
                        nc.vector.scalar_tensor_tensor(
                            scr[:], pooled_ps[:], 1.0, ct["clsw"][:],
                            op0=ALU.mult, op1=ALU.mult,
                            accum_out=dots[0:1, g:g + 1])

            nc.vector.tensor_scalar(out_sb[:], dots[:], ct["clsb"][:], None,
                                    op0=ALU.add)
            nc.sync.dma_start(out_d[:], out_sb[:])

    nc.compile()
    return nc


def _prep_host(inputs):
    """Build the host-side constant tensors and per-core xprep arrays."""
    f32 = np.float32
    cst = {}
    w1 = np.asarray(inputs["conv1_w"], f32)
    wc1 = np.zeros((128, 32), f32)
    for j in range(4):
        for k in range(3):
            wc1[32 * j + k, :] = w1[:, 0, k]
    cst["wc1"] = wc1.astype(_BF16)
    w2 = np.asarray(inputs["conv2_w"], f32)
    w3 = np.asarray(inputs["conv3_w"], f32)
    w4 = np.asarray(inputs["conv4_w"], f32)
    for k in range(3):
        a = np.zeros((128, 64), f32)
        for j in range(4):
            a[32 * j:32 * j + 32, :] = w2[:, :, k].T
        cst[f"wc2k{k}"] = a.astype(_BF16)
        a = np.zeros((128, 128), f32)
        a[0:64, :] = w3[:, :, k].T
        a[64:128, :] = w3[:, :, k].T
        cst[f"wc3k{k}"] = a.astype(_BF16)
        for m in range(2):
            cst[f"wc4k{k}m{m}"] = w4[128 * m:128 * m + 128, :, k].T.copy().astype(_BF16)
    b1 = np.asarray(inputs["conv1_b"], f32)
    b2 = np.asarray(inputs["conv2_b"], f32)
    cst["bias1"] = np.tile(b1, 4).reshape(128, 1).astype(f32)
    cst["bias2"] = np.tile(b2, 2).reshape(128, 1).astype(f32)
    cst["bias3"] = np.asarray(inputs["conv3_b"], f32).reshape(128, 1)
    b4 = np.asarray(inputs["conv4_b"], f32)
    cst["bias4a"] = b4[0:128].reshape(128, 1).copy()
    cst["bias4b"] = b4[128:256].reshape(128, 1).copy()
    fcw = np.asarray(inputs["fc_w"], f32)
    cst["fcwT0"] = (fcw[:, 0:128].T / L).astype(f32).copy()
    cst["fcwT1"] = (fcw[:, 128:256].T / L).astype(f32).copy()
    cst["fcb_bc"] = np.broadcast_to(np.asarray(inputs["fc_b"], f32), (128, 256)).copy()
    for l in range(3):
        W = np.asarray(inputs[f"gat{l+1}_w"], f32)      # [256 out, 256 in]
        As = np.asarray(inputs[f"gat{l+1}_as"], f32)[0]  # [4, 64]
        Ad = np.asarray(inputs[f"gat{l+1}_ad"], f32)[0]
        for t in range(2):
            cst[f"wtT{l}t{t}"] = W[:, 128 * t:128 * t + 128].T.copy().astype(_BF16)
        was = np.zeros((256, 8), f32)
        for hh in range(4):
            was[:, hh] = W[64 * hh:64 * hh + 64, :].T @ As[hh]
            was[:, 4 + hh] = W[64 * hh:64 * hh + 64, :].T @ Ad[hh]
        cst[f"wasad{l}t0"] = was[0:128].astype(_BF16)
        cst[f"wasad{l}t1"] = was[128:256].astype(_BF16)
        cst[f"gatb_bc{l}"] = np.broadcast_to(
            np.asarray(inputs[f"gat{l+1}_b"], f32), (128, 256)).copy()
        cst[f"nb_bc{l}"] = np.broadcast_to(
            np.asarray(inputs[f"norm{l+1}_b"], f32), (128, 256)).copy()
        cst[f"msrow{l}"] = np.asarray(inputs[f"norm{l+1}_ms"], f32).reshape(1, 256).copy()
        cst[f"grow{l}"] = np.asarray(inputs[f"norm{l+1}_g"], f32).reshape(1, 256).copy()
    ei = np.asarray(inputs["edge_index"])
    src, dst = ei[0], ei[1]
    cnt = np.zeros((N, N), f32)
    np.add.at(cnt, (dst, src), 1.0)
    cnt += np.eye(N, dtype=f32)
    cst["cntT"] = cnt.T.copy()
    cst["ones_col"] = np.full((128, 1), 1.0 / N, f32)
    cst["ones_row_f"] = np.ones((1, 128), f32)
    cst["ones_row_bf"] = np.ones((1, 128), _BF16)
    cst["ones_col_bf"] = np.ones((128, 1), _BF16)
    cst["ident"] = np.eye(128, dtype=f32)
    cst["clsw"] = np.asarray(inputs["cls_w"], f32).reshape(1, 256).copy()
    cst["clsb"] = np.asarray(inputs["cls_b"], f32).reshape(1, 1).copy()

    # xprep: [core][g, 32*j+k, (t+1)*GC + n] = x[b, t+k-1, 32*j+n]
    x = np.asarray(inputs["x"], f32)   # [B, L, N]
    ts = np.arange(L)
    xprep_all = []
    for core in range(NCORES):
        xp = np.zeros((GPC, 128, TC), f32)
        for g in range(GPC):
            b = core * GPC + g
            for k in range(3):
                st = ts + k - 1
                valid = (st >= 0) & (st < L)
                for j in range(4):
                    blk = np.zeros((L, GC), f32)
                    blk[valid] = x[b][st[valid]][:, 32 * j:32 * j + 32]
                    xp[g, 32 * j + k, GC:GC + L * GC] = blk.reshape(-1)
        xprep_all.append(xp.astype(_BF16))
    return cst, xprep_all


def kernel(**inputs):
    from concourse.bass_utils import run_bass_kernel_spmd

    if "nc" not in _cache:
        _cache["nc"] = _build_program()
    nc = _cache["nc"]

    cst, xprep_all = _prep_host(inputs)
    in_maps = []
    for core in range(NCORES):
        m = dict(cst)
        m["xprep"] = xprep_all[core]
        in_maps.append(m)
    res = run_bass_kernel_spmd(nc, in_maps, list(range(NCORES)))
    out = np.zeros((B, 1), np.float32)
    for core in range(NCORES):
        o = np.asarray(res.results[core]["out"]).reshape(GPC)
        for g in range(GPC):
            out[core * GPC + g, 0] = o[g]
    return out
